# revision 2
# baseline (speedup 1.0000x reference)
"""Bahdanau-attention decoder cell (GRU-style) on 8 Trainium2 NeuronCores.

Sharding: data-parallel over batch. Each of the 8 cores processes 8 of the
64 examples; all weight matrices are replicated. No collectives needed.

v2 redesign: both encoder layouts come straight from DRAM (the transposed
copy is built on the host), eliminating the on-chip PE-transpose +
psum->SBUF copy pipeline that dominated DVE/ACT in v1.

Per-core pipeline (per example b), all fp8 on-chip:
  1. DMA enc[b]   (1024x2048 f32) -> SBUF h8  [128p, 8k, l] fp8 (2 halves)
     DMA encT[b]  (2048x1024 f32) -> SBUF hT8 [128p, 8t, h2] fp8 (2 halves)
     (SWDGE casts f32->fp8 during the transfer.)
  2. scores psum[128a, 2lc, 512] = DoubleRow fp8 matmuls (lhsT = 64*Ua.T)
  3. v[:, m, lc, :] = tanh(psum/64 + decT[:, m, b])  one ACT instr per
     (m, lc-pair), per-partition bias -> fp8
  4. energies eps[1,512] += va_col DR matvecs over m-pairs; e row copy-out
     (split ACT/DVE)
  5. softmax: e -> columns via K=1 matmuls, exp (accum_out -> S),
     w8 = w * (256/S) -> fp8
  6. context ct[1, 1024] += DR matvecs with lhsT = w8 lt-pairs,
     rhs = hT8 tiles straight from DMA; c row = ct/256 -> bf16 (DVE)
Then batched over the core's 8 examples: GRU gates r,z,s_prop as
[hout,b]-major bf16 matmuls (weights pre-transposed+packed on host),
final combine in f32, PE-transpose back to [b,hout], DMA out.
"""

import numpy as np
import ml_dtypes

import concourse.bass as bass
import concourse.tile as tile
from concourse import bacc
from concourse import mybir
from concourse.bass_utils import run_bass_kernel_spmd
from concourse.masks import make_identity

F32 = mybir.dt.float32
BF16 = mybir.dt.bfloat16
FP8 = mybir.dt.float8e4
AF = mybir.ActivationFunctionType
DR = mybir.MatmulPerfMode.DoubleRow

N_CORES = 8
B, IN, H, A, L = 64, 512, 512, 512, 2048
H2 = 2 * H
BL = B // N_CORES  # examples per core
KA = H2 // 128     # k-tiles over the 2H contraction dim
LT = L // 128      # l-tiles over the source-position dim

UA_SCALE = 64.0    # Ua pre-scale so fp8 values stay out of subnormals
W_EXP = 16.0       # exp pre-scale: w8 = W_EXP*exp(e) stays in fp8 range
                   # (e ~ N(0, ~0.3) here, so W_EXP*exp(e) ~ [3, 50] << 448)
GW_SCALE = 32.0    # GRU gate-weight pre-scale for fp8 storage


def build_decoder_cell(n_ex: int = BL):
    nc = bacc.Bacc(None, target_bir_lowering=False, debug=True)

    x16 = nc.declare_dram_parameter("x16", [n_ex, IN], BF16, isOutput=False)
    sp16 = nc.declare_dram_parameter("sp16", [n_ex, H], BF16, isOutput=False)
    sp32 = nc.declare_dram_parameter("sp32", [n_ex, H], F32, isOutput=False)
    enc = nc.declare_dram_parameter("enc", [n_ex, H2, L], F32, isOutput=False)
    encT = nc.declare_dram_parameter("encT", [n_ex, L, H2], F32, isOutput=False)
    uaT = nc.declare_dram_parameter("uaT", [128, KA * A], FP8, isOutput=False)
    waT = nc.declare_dram_parameter("waT", [128, 4 * A], BF16, isOutput=False)
    # r-gate weights tolerate fp8 (their error is damped through tanh'
    # and z); z/s-gate weights feed the output directly -> bf16
    wrT = nc.declare_dram_parameter("wrT", [128, 4 * H], FP8, isOutput=False)
    wzT = nc.declare_dram_parameter("wzT", [128, 4 * H], BF16, isOutput=False)
    wsT = nc.declare_dram_parameter("wsT", [128, 4 * H], BF16, isOutput=False)
    urT = nc.declare_dram_parameter("urT", [128, 4 * H], FP8, isOutput=False)
    uzT = nc.declare_dram_parameter("uzT", [128, 4 * H], BF16, isOutput=False)
    usT = nc.declare_dram_parameter("usT", [128, 4 * H], BF16, isOutput=False)
    crT = nc.declare_dram_parameter("crT", [128, KA * H], FP8, isOutput=False)
    czT = nc.declare_dram_parameter("czT", [128, KA * H], BF16, isOutput=False)
    csT = nc.declare_dram_parameter("csT", [128, KA * H], BF16, isOutput=False)
    va_c = nc.declare_dram_parameter("va_c", [128, 32], FP8, isOutput=False)
    y = nc.declare_dram_parameter("y", [n_ex, H], F32, isOutput=True)

    enc_t = enc[:].rearrange("e (k p) l -> e p k l", p=128)
    encT_t = encT[:].rearrange("e (t p) h -> e p t h", p=128)

    with tile.TileContext(nc) as tc:
        with (
            tc.tile_pool(name="singles", bufs=1) as singles,
            tc.tile_pool(name="hpool", bufs=6) as hpool,
            tc.tile_pool(name="htpool", bufs=6) as htpool,
            tc.tile_pool(name="vpool", bufs=3) as vpool,
            tc.tile_pool(name="smpool", bufs=3) as smpool,
            tc.tile_pool(name="ps_mm", bufs=2, space="PSUM") as ps_mm,
            tc.tile_pool(name="ps_e", bufs=1, space="PSUM") as ps_e,
            tc.tile_pool(name="ps_t", bufs=1, space="PSUM") as ps_t,
            tc.tile_pool(name="ps_c", bufs=1, space="PSUM") as ps_c,
        ):
            # ---- one-time setup ----
            id128f = singles.tile([128, 128], F32)
            make_identity(nc, id128f)
            idb = singles.tile([n_ex, n_ex], BF16)
            make_identity(nc, idb)
            idbf = singles.tile([n_ex, n_ex], F32)
            make_identity(nc, idbf)
            oneb = singles.tile([1, 1], BF16)
            nc.vector.memset(oneb, 1.0)
            ones_col = singles.tile([128, 1], F32)
            nc.vector.memset(ones_col, 1.0)

            uaT_sb = singles.tile([128, KA, A], FP8)
            nc.sync.dma_start(out=uaT_sb, in_=uaT[:].rearrange("p (k a) -> p k a", k=KA))
            waT_sb = singles.tile([128, 4 * A], BF16)
            nc.sync.dma_start(out=waT_sb, in_=waT[:])
            # gate weights are only needed by the GRU at the end; their DMAs
            # are issued after the per-example loop so they don't delay the
            # first encoder loads on the shared DMA engines.
            gate_w = {}
            gate_dram = {"wrT": wrT, "wzT": wzT, "wsT": wsT,
                         "urT": urT, "uzT": uzT, "usT": usT}
            for nm in gate_dram:
                dt = FP8 if nm in ("wrT", "urT") else BF16
                gate_w[nm] = singles.tile([128, 4 * H], dt, name=nm + "_sb")
            for nm, dram in [("crT", crT), ("czT", czT), ("csT", csT)]:
                gate_dram[nm] = dram
                dt = FP8 if nm == "crT" else BF16
                gate_w[nm] = singles.tile([128, KA * H], dt, name=nm + "_sb")
            va_sb = singles.tile([128, 2, 16], FP8)
            nc.sync.dma_start(out=va_sb, in_=va_c[:].rearrange(
                "p (two j) -> p two j", two=2))

            x16_sb = singles.tile([n_ex, IN], BF16)
            nc.sync.dma_start(out=x16_sb, in_=x16[:])
            sp16_sb = singles.tile([n_ex, H], BF16)
            nc.sync.dma_start(out=sp16_sb, in_=sp16[:])
            sp32_sb = singles.tile([n_ex, H], F32)
            nc.sync.dma_start(out=sp32_sb, in_=sp32[:])

            # transpose x / sprev to [feat-part, k, b]
            xT_sb = singles.tile([128, 4, n_ex], BF16)
            spT16_sb = singles.tile([128, 4, n_ex], BF16)
            spT32_sb = singles.tile([128, 4, n_ex], F32)
            for j in range(4):
                pst = ps_t.tile([128, 512], BF16, tag="ps_t", name="pst_x")
                nc.tensor.transpose(pst[:, :n_ex], x16_sb[:, j * 128:(j + 1) * 128], idb)
                nc.vector.tensor_copy(xT_sb[:, j, :], pst[:, :n_ex])
                pst2 = ps_t.tile([128, 512], BF16, tag="ps_t", name="pst_s")
                nc.tensor.transpose(pst2[:, :n_ex], sp16_sb[:, j * 128:(j + 1) * 128], idb)
                nc.vector.tensor_copy(spT16_sb[:, j, :], pst2[:, :n_ex])
                pst3 = ps_t.tile([128, 512], F32, tag="ps_t", name="pst_s32")
                nc.tensor.transpose(pst3[:, :n_ex], sp32_sb[:, j * 128:(j + 1) * 128], idbf)
                nc.vector.tensor_copy(spT32_sb[:, j, :], pst3[:, :n_ex])

            cT_sb = singles.tile([128, KA, n_ex], BF16)

            # decT[a, b] = (sprev @ Wa.T).T
            decT_sb = singles.tile([128, 4, n_ex], F32)
            for m in range(4):
                ps = ps_mm.tile([128, 2, 512], F32, tag="ps_mm", name="ps_dec")
                for k in range(4):
                    nc.tensor.matmul(
                        ps[:, 0, :n_ex],
                        lhsT=waT_sb[:, k * A + m * 128:k * A + (m + 1) * 128],
                        rhs=spT16_sb[:, k, :],
                        start=(k == 0), stop=(k == 3),
                    )
                nc.vector.tensor_copy(decT_sb[:, m, :], ps[:, 0, :n_ex])

            # exp bias: ln(W_EXP) so the exp emits W_EXP * exp(e) directly
            lnw_sb = singles.tile([128, 1], F32)
            nc.vector.memset(lnw_sb, float(np.log(W_EXP)))

            # ---- per-example attention, software-pipelined ----
            # S1(b): DMAs + scores matmuls + tanh (PE/ACT streaming work)
            # S2(b): energies / softmax / context / scatter (latency chain)
            # Issue order S1(0), S1(1), S2(0), S1(2), S2(1), ... so the PE
            # queue always has S1 work ready while S2's cross-engine chain
            # resolves.
            stash = {}

            def stage1(b):
                h_halves = []
                ht_halves = []
                for hf in range(2):
                    ht = hpool.tile([128, KA, L // 2], FP8, tag="h",
                                    name=f"h_{b}_{hf}")
                    nc.gpsimd.dma_start(
                        out=ht, in_=enc_t[b][:, :, hf * (L // 2):(hf + 1) * (L // 2)])
                    h_halves.append(ht)
                    htt = htpool.tile([128, LT // 2, H2], FP8, tag="ht",
                                      name=f"hT_{b}_{hf}")
                    nc.gpsimd.dma_start(
                        out=htt, in_=encT_t[b][:, hf * (LT // 2):(hf + 1) * (LT // 2), :])
                    ht_halves.append(htt)

                # scores + tanh: v[p, m, lc, n]
                v_sb = vpool.tile([128, 4, 4, 512], FP8, tag="v", name=f"v_{b}")
                for m in range(4):
                    for lcp in range(2):
                        ps = ps_mm.tile([128, 2, 512], F32, tag="ps_mm",
                                        name=f"ps_s{b}_{m}_{lcp}")
                        for lc2 in range(2):
                            for ks in range(KA // 2):
                                nc.tensor.matmul(
                                    ps[:, lc2, :],
                                    lhsT=uaT_sb[:, 2 * ks:2 * ks + 2,
                                                m * 128:(m + 1) * 128],
                                    rhs=h_halves[lcp][:, 2 * ks:2 * ks + 2,
                                                      lc2 * 512:(lc2 + 1) * 512],
                                    start=(ks == 0), stop=(ks == KA // 2 - 1),
                                    perf_mode=DR,
                                )
                        nc.scalar.activation(
                            v_sb[:, m, 2 * lcp:2 * lcp + 2, :], ps, AF.Tanh,
                            bias=decT_sb[:, m, b:b + 1], scale=1.0 / UA_SCALE)
                stash[b] = (v_sb, ht_halves)

            def stage2(b):
                v_sb, ht_halves = stash.pop(b)
                # energies, directly in column form (at UA_SCALE; undone
                # inside the exp): et[:, col(t)] = sum_m v_m[:, tile t].T @ va_m
                et_ps = ps_t.tile([128, 512], F32, tag="ps_t", name=f"etps_{b}")
                for t in range(16):
                    col = (t % 2) * 16 + t // 2
                    lc, c = t // 4, t % 4
                    for q in range(2):
                        nc.tensor.matmul(
                            et_ps[:, col:col + 1],
                            lhsT=v_sb[:, 2 * q:2 * q + 2, lc, c * 128:(c + 1) * 128],
                            rhs=va_sb[:, :, q:q + 1],
                            start=(q == 0), stop=(q == 1),
                            perf_mode=DR)
                et_v = et_ps[:, :32].rearrange("p (two j) -> p two j", two=2)
                # [128, 2, 16] (32B/partition) keeps the Ldweights AP legal;
                # only cols :8 are used
                wT_sb = smpool.tile([128, 2, 16], FP8, tag="wT", name=f"wT_{b}")
                psum_sb = smpool.tile([128, 1], F32, tag="S", name=f"S_{b}")
                nc.scalar.activation(wT_sb[:, :, :8], et_v[:, :, :8], AF.Exp,
                                     bias=lnw_sb, scale=1.0 / UA_SCALE,
                                     accum_out=psum_sb)
                stot_ps = ps_e.tile([1, 512], F32, tag="ps_e", name=f"stot_{b}")
                nc.tensor.matmul(stot_ps[:, :1], lhsT=psum_sb, rhs=ones_col,
                                 start=True, stop=True)
                invs_sb = smpool.tile([1, 1], F32, tag="invS", name=f"invS_{b}")
                nc.vector.reciprocal(invs_sb, stot_ps[:, :1])

                # context: ct[1, H2] += DR matvecs straight off the DMA'd hT
                ct_ps = ps_c.tile([1, H2], F32, tag="ps_c", name=f"ctps_{b}")
                for j in range(8):
                    for half in range(2):
                        nc.tensor.matmul(
                            ct_ps[:, half * 512:(half + 1) * 512],
                            lhsT=wT_sb[:, :, j:j + 1],
                            rhs=ht_halves[j // 4][:, (j % 4) * 2:(j % 4) * 2 + 2,
                                                  half * 512:(half + 1) * 512],
                            start=(j == 0), stop=(j == 7),
                            perf_mode=DR,
                        )
                c_row_sb = smpool.tile([1, H2], BF16, tag="crow", name=f"crow_{b}")
                nc.vector.tensor_scalar_mul(c_row_sb, in0=ct_ps,
                                            scalar1=invs_sb)

                # scatter c into column-major cT_sb[:, j, b] via K=1 matmuls
                ctt_ps = ps_t.tile([128, 512], F32, tag="ps_t", name=f"cttps_{b}")
                for j in range(KA):
                    nc.tensor.matmul(ctt_ps[:, j:j + 1],
                                     lhsT=c_row_sb[:, j * 128:(j + 1) * 128],
                                     rhs=oneb, start=True, stop=True)
                nc.vector.tensor_copy(cT_sb[:, :, b:b + 1], ctt_ps[:, :KA])

            for b in range(n_ex):
                stage1(b)
                if b >= 1:
                    stage2(b - 1)
            stage2(n_ex - 1)

            # gate-weight loads, issued only now (see note above)
            for nm, dram in gate_dram.items():
                nc.sync.dma_start(out=gate_w[nm], in_=dram[:])

            # ---- batched GRU over the core's examples ----
            # fp8 weights (x GW_SCALE) against bf16 activations; all 4 hout
            # tiles share one psum region so each gate is a single
            # activation / elementwise instruction over [128, 4, n_ex].
            def gate_psum(wname, uname, cname, u_rhs, name):
                """psum[hout-tile m, b] = W.T@xT + U.T@u_rhs + C.T@cT."""
                ps = ps_mm.tile([128, 2, 512], F32, tag="ps_mm", name=name)
                g = ps[:, 0, :4 * n_ex].rearrange("p (m b) -> p m b", m=4)
                wt, ut, ct = gate_w[wname], gate_w[uname], gate_w[cname]
                for m in range(4):
                    for k in range(4):
                        nc.tensor.matmul(
                            g[:, m, :],
                            lhsT=wt[:, k * H + m * 128:k * H + (m + 1) * 128],
                            rhs=xT_sb[:, k, :], start=(k == 0), stop=False)
                    for k in range(4):
                        nc.tensor.matmul(
                            g[:, m, :],
                            lhsT=ut[:, k * H + m * 128:k * H + (m + 1) * 128],
                            rhs=u_rhs[:, k, :], start=False, stop=False)
                    for k in range(KA):
                        nc.tensor.matmul(
                            g[:, m, :],
                            lhsT=ct[:, k * H + m * 128:k * H + (m + 1) * 128],
                            rhs=cT_sb[:, k, :], start=False, stop=(k == KA - 1))
                return g

            # sigmoid(a) = 0.5*(1 + tanh(a/2)) keeps every activation in the
            # exp/tanh/copy table set -> no act-table reload before the GRU.
            ALU = mybir.AluOpType
            sp_half = singles.tile([128, 4, n_ex], F32)
            nc.vector.tensor_scalar_mul(sp_half, in0=spT32_sb, scalar1=0.5)

            g_z = gate_psum("wzT", "uzT", "czT", spT16_sb, "ps_z")
            thz_sb = singles.tile([128, 4, n_ex], F32)
            nc.scalar.activation(thz_sb, g_z, AF.Tanh, scale=0.5)

            g_r = gate_psum("wrT", "urT", "crT", spT16_sb, "ps_r")
            thr_sb = singles.tile([128, 4, n_ex], F32)
            nc.scalar.activation(thr_sb, g_r, AF.Tanh, scale=0.5 / GW_SCALE)
            # r*sprev = (1 + tanh) * 0.5*sprev
            rs16_sb = singles.tile([128, 4, n_ex], BF16)
            nc.vector.scalar_tensor_tensor(rs16_sb, in0=thr_sb, scalar=1.0,
                                           in1=sp_half, op0=ALU.add,
                                           op1=ALU.mult)

            g_s = gate_psum("wsT", "usT", "csT", rs16_sb, "ps_p")
            sp_prop = singles.tile([128, 4, n_ex], F32)
            nc.scalar.activation(sp_prop, g_s, AF.Tanh)

            # out = sprev + z*(s_prop - sprev), z = 0.5*(1 + thz)
            outT_sb = singles.tile([128, 4, n_ex], F32)
            d_sb = singles.tile([128, 4, n_ex], F32)
            q_sb = singles.tile([128, 4, n_ex], F32)
            nc.vector.tensor_sub(d_sb, sp_prop, spT32_sb)
            nc.vector.scalar_tensor_tensor(q_sb, in0=thz_sb, scalar=1.0,
                                           in1=d_sb, op0=ALU.add, op1=ALU.mult)
            nc.vector.scalar_tensor_tensor(outT_sb, in0=q_sb, scalar=0.5,
                                           in1=spT32_sb, op0=ALU.mult,
                                           op1=ALU.add)

            o_ps = ps_t.tile([128, 512], F32, tag="ps_t", name="o_ps")
            for m in range(4):
                nc.tensor.transpose(o_ps[:n_ex, m * 128:(m + 1) * 128],
                                    outT_sb[:, m, :], id128f)
            y_sb = singles.tile([n_ex, H], F32)
            nc.scalar.copy(y_sb, o_ps[:n_ex, :])
            nc.sync.dma_start(out=y[:], in_=y_sb)

    nc.compile()
    return nc


def _pack(wT: np.ndarray) -> np.ndarray:
    """[K, M] (K = contraction) -> [128, (K//128)*M] with slice
    [:, k*M + j] == wT[k*128 + p, j]."""
    K, M = wT.shape
    return np.ascontiguousarray(
        wT.reshape(K // 128, 128, M).transpose(1, 0, 2).reshape(128, -1))


def _pack_va(va: np.ndarray) -> np.ndarray:
    out = np.zeros((128, 2, 16), dtype=ml_dtypes.float8_e4m3fn)
    for q in range(2):
        for ko in range(2):
            out[:, ko, q] = (va[(2 * q + ko) * 128:(2 * q + ko + 1) * 128]
                             * UA_SCALE).astype(ml_dtypes.float8_e4m3fn)
    return out.reshape(128, 32)


_BUILT = {}


def _get_nc(n_ex: int):
    if n_ex not in _BUILT:
        _BUILT[n_ex] = build_decoder_cell(n_ex)
    return _BUILT[n_ex]


LAST_RESULTS = None


def kernel(x, sprev, encoder_hiddens, Ws, Wz, Wr, Us, Uz, Ur,
           Cs, Cz, Cr, bs, bz, br, va, Wa, Ua, _trace=False,
           _cores=None) -> np.ndarray:
    global LAST_RESULTS
    bf = ml_dtypes.bfloat16
    f8 = ml_dtypes.float8_e4m3fn
    nc = _get_nc(BL)

    wmap = {
        "uaT": _pack((Ua.T * UA_SCALE).astype(f8)),
        "waT": _pack(Wa.T.astype(bf)),
        "wrT": _pack((Wr.T * GW_SCALE).astype(f8)),
        "wzT": _pack(Wz.T.astype(bf)),
        "wsT": _pack(Ws.T.astype(bf)),
        "urT": _pack((Ur.T * GW_SCALE).astype(f8)),
        "uzT": _pack(Uz.T.astype(bf)),
        "usT": _pack(Us.T.astype(bf)),
        "crT": _pack((Cr.T * GW_SCALE).astype(f8)),
        "czT": _pack(Cz.T.astype(bf)),
        "csT": _pack(Cs.T.astype(bf)),
        "va_c": _pack_va(va),
    }
    cores = list(range(N_CORES)) if _cores is None else _cores
    in_maps = []
    for i in cores:
        sl = slice(i * BL, (i + 1) * BL)
        enc_sl = np.ascontiguousarray(encoder_hiddens[sl])
        in_maps.append({
            "x16": x[sl].astype(bf),
            "sp16": sprev[sl].astype(bf),
            "sp32": np.ascontiguousarray(sprev[sl]),
            "enc": enc_sl,
            "encT": np.ascontiguousarray(enc_sl.transpose(0, 2, 1)),
            **wmap,
        })
    res = run_bass_kernel_spmd(nc, in_maps, core_ids=list(range(len(cores))),
                               trace=_trace)
    LAST_RESULTS = res
    return np.concatenate([res.results[i]["y"] for i in range(len(cores))],
                          axis=0)
